# revision 29
# baseline (speedup 1.0000x reference)
"""Trainium2 Bass kernel for nn_Attention_8366596292664.

Dense transformer block: qkv proj -> RoPE -> GQA causal attention ->
out proj -> RMSNorm.  B=4, S=2048, H=2048, 16 heads (hd=128), 4 KV heads.

Sharding: 8 cores = (4 batches) x (2 interleaved query-row parities).
Core (b, par) computes the full block for query rows {par, par+2, ...} of
batch b.  Interleaving the query rows by parity makes the causal structure
identical on every core, so one SPMD program serves all 8 cores; the
parity enters only through the data (a 1-column roll of x^T, cos/sin
tables, and the output row scatter).

v2 changes vs v1:
  - Trapezoid causal attention: scores/exp/mask/probsV/denominator all
    run on the live column sub-range of each key tile.  The diagonal
    64-column sliver mask is self-similar across tiles, so the mask
    table shrinks to one [128, 64] tile.
  - Startup: x^T loaded in four column blocks + k-proj iterates blocks
    outermost, and dummy warm-up matmuls run during the DMA wait so the
    PE HAM clock-gate is released before real work arrives.
  - w_proj prefetched during attention (kills the phase-3 entry stall).
"""

import numpy as np
import ml_dtypes

BF16 = ml_dtypes.bfloat16

# ---------------------------------------------------------------- config
P = 128          # partitions
HD = 128         # head dim
HH = HD // 2     # rope half
G = 4            # GQA group size

B = 4
S = 2048
H = 2048
N_CORES = 8

NH = H // HD          # 16 q heads
NKV = NH // G         # 4 kv heads
KVC = NKV * HD        # 512 kv columns
HT = H // P           # 16 h-tiles (contraction tiles)
S_LOC = S // 2        # 1024 local q rows per core
IT = 512              # i-tile (queries per score tile, = 1 psum bank fp32)
NT_I = S_LOC // IT    # 2 i-slots
JB = 8                # j-tiles per 1024-key band
OT = 512              # output-proj column tile
NO = H // OT          # 4
NWARM = 260           # PE warm-up matmuls: cover the ~28us startup DMA

RMS_EPS = 1e-6
SCALE = 1.0 / float(np.sqrt(np.float32(HD)))

_CACHE = {}


# ---------------------------------------------------------------- device IR
def _build_nc():
    from contextlib import ExitStack

    import concourse.bacc as bacc
    import concourse.mybir as mybir
    import concourse.tile as tile

    dt = mybir.dt
    AF = mybir.ActivationFunctionType

    nc = bacc.Bacc("TRN2", target_bir_lowering=False, debug=False)

    # host-packed so every DMA row is contiguous per partition (no small
    # packets): wq[f][p] = 2KB, wk/wv[p] = 8KB, wp[p] = 64KB, xt[p] = 64KB
    xt_d = nc.dram_tensor("xt", [P, HT * S], dt.bfloat16, kind="ExternalInput")
    wq_d = nc.dram_tensor("wq", [NH, P, HT * HD], dt.bfloat16, kind="ExternalInput")
    wk_d = nc.dram_tensor("wk", [P, NKV * HT * HD], dt.bfloat16, kind="ExternalInput")
    wv_d = nc.dram_tensor("wv", [P, HT * KVC], dt.bfloat16, kind="ExternalInput")
    wp_d = nc.dram_tensor("wp", [P, HT * H], dt.bfloat16, kind="ExternalInput")
    qcos_d = nc.dram_tensor("qcos", [P, S_LOC], dt.bfloat16, kind="ExternalInput")
    qsin_d = nc.dram_tensor("qsin", [P, S_LOC], dt.bfloat16, kind="ExternalInput")
    kcos_d = nc.dram_tensor("kcos", [P, S], dt.bfloat16, kind="ExternalInput")
    ksin_d = nc.dram_tensor("ksin", [P, S], dt.bfloat16, kind="ExternalInput")
    msl_d = nc.dram_tensor("msl", [P, 64], dt.bfloat16, kind="ExternalInput")
    out_d = nc.dram_tensor("out", [S_LOC, H], dt.float32, kind="ExternalOutput")

    with tile.TileContext(nc) as tc, ExitStack() as body:
        const = body.enter_context(tc.tile_pool(name="const", bufs=1))
        qcos = const.tile([P, S_LOC], dt.bfloat16)
        qsin = const.tile([P, S_LOC], dt.bfloat16)
        kcos = const.tile([P, S], dt.bfloat16)
        ksin = const.tile([P, S], dt.bfloat16)
        onesm = const.tile([P, P], dt.bfloat16)
        nc.vector.memset(onesm[:], 1.0)
        msl = const.tile([P, 64], dt.bfloat16)
        epsb = const.tile([P, 1], dt.float32)
        nc.vector.memset(epsb[:], RMS_EPS)
        nc.sync.dma_start(msl[:], msl_d.ap())
        nc.sync.dma_start(qcos[:], qcos_d.ap())
        nc.sync.dma_start(qsin[:], qsin_d.ap())
        nc.sync.dma_start(kcos[:], kcos_d.ap())
        nc.sync.dma_start(ksin[:], ksin_d.ap())

        s_act = body.enter_context(ExitStack())
        act = s_act.enter_context(tc.tile_pool(name="act", bufs=1))
        qT = act.tile([P, NH * S_LOC], dt.bfloat16)
        kT = act.tile([P, NKV * S], dt.bfloat16)
        vv = act.tile([P, (S // P) * KVC], dt.bfloat16)

        def rope_evict(rpool, ps, dst_lo, dst_hi, cs, sn):
            # dst_lo = ps_lo*cos - ps_hi*sin ; dst_hi = ps_hi*cos + ps_lo*sin
            stg = rpool.tile([P, IT], dt.bfloat16, name="rstg")
            nc.scalar.activation(stg[:], ps[:], AF.Copy)
            t1 = rpool.tile([HH, IT], dt.bfloat16, name="rt1")
            t2 = rpool.tile([HH, IT], dt.bfloat16, name="rt2")
            nc.vector.tensor_mul(t1[:], stg[0:HH, :], cs[0:HH, :])
            nc.vector.tensor_mul(t2[:], stg[HH:P, :], sn[HH:P, :])
            nc.vector.tensor_sub(dst_lo, t1[:], t2[:])
            nc.vector.tensor_mul(t1[:], stg[HH:P, :], cs[HH:P, :])
            nc.vector.tensor_mul(t2[:], stg[0:HH, :], sn[0:HH, :])
            nc.vector.tensor_add(dst_hi, t1[:], t2[:])

        # ---------------- phase 1: qkv projection + rope --------------
        with ExitStack() as ph1:
            xp = ph1.enter_context(tc.tile_pool(name="xp", bufs=1))
            xt = xp.tile([P, HT * S], dt.bfloat16)

            wkp = ph1.enter_context(tc.tile_pool(name="wkp", bufs=1))
            wk = wkp.tile([P, NKV * HT * HD], dt.bfloat16)
            nc.sync.dma_start(wk[:], wk_d.ap())
            # x^T in four column blocks so the first k-proj tile can start
            # after ~3MB instead of ~13MB of DMA.
            xt_r = xt[:].rearrange("p (t s) -> p t s", t=HT)
            xd_r = xt_d.ap().rearrange("p (t s) -> p t s", t=HT)
            for sc in range(S // IT):
                nc.sync.dma_start(
                    xt_r[:, :, sc * IT : (sc + 1) * IT],
                    xd_r[:, :, sc * IT : (sc + 1) * IT],
                )

            wvp = ph1.enter_context(tc.tile_pool(name="wvp", bufs=1))
            wv = wvp.tile([P, HT * KVC], dt.bfloat16)
            nc.sync.dma_start(wv[:], wv_d.ap())

            wqp = ph1.enter_context(tc.tile_pool(name="wqp", bufs=2))
            rp1 = ph1.enter_context(tc.tile_pool(name="rp1", bufs=2))
            psq = ph1.enter_context(tc.tile_pool(name="psq", bufs=4, space="PSUM"))
            psk = ph1.enter_context(tc.tile_pool(name="psk", bufs=2, space="PSUM"))
            psv = ph1.enter_context(tc.tile_pool(name="psv", bufs=2, space="PSUM"))

            # PE warm-up: release the HAM clock gate while startup DMAs
            # are in flight (no data deps -> runs immediately).
            wps = psk.tile([P, IT], dt.float32, name="kps")
            for _ in range(NWARM):
                nc.tensor.matmul(wps[:, 0:P], onesm[:], onesm[:], start=True, stop=True)

            # k projection (sc-block outermost: only needs xt block sc) + rope
            for sc in range(S // IT):
                for fk in range(NKV):
                    ps = psk.tile([P, IT], dt.float32, name="kps")
                    for h in range(HT):
                        nc.tensor.matmul(
                            ps[:],
                            wk[:, fk * H + h * HD : fk * H + (h + 1) * HD],
                            xt[:, h * S + sc * IT : h * S + (sc + 1) * IT],
                            start=(h == 0),
                            stop=(h == HT - 1),
                        )
                    c0 = fk * S + sc * IT
                    rope_evict(
                        rp1, ps,
                        kT[0:HH, c0 : c0 + IT], kT[HH:P, c0 : c0 + IT],
                        kcos[:, sc * IT : (sc + 1) * IT],
                        ksin[:, sc * IT : (sc + 1) * IT],
                    )

            # v projection (natural [s, f] layout)
            for sv in range(S // P):
                ps = psv.tile([P, KVC], dt.float32, name="vps")
                for h in range(HT):
                    nc.tensor.matmul(
                        ps[:],
                        xt[:, h * S + sv * P : h * S + (sv + 1) * P],
                        wv[:, h * KVC : (h + 1) * KVC],
                        start=(h == 0),
                        stop=(h == HT - 1),
                    )
                nc.scalar.activation(
                    vv[:, sv * KVC : (sv + 1) * KVC], ps[:], AF.Copy
                )

            # q projection (local rows, stride-2 reads of x^T) + rope.
            for fq in range(NH):
                wq = wqp.tile([P, HT * HD], dt.bfloat16, name="wqt")
                if fq == 0:
                    nc.vector.tensor_scalar_mul(wq[:, 0:1], wv[:, 0:1], 1.0)
                nc.sync.dma_start(wq[:], wq_d.ap()[fq])
                pss = [psq.tile([P, IT], dt.float32, name="qps") for _ in range(NT_I)]
                for h in range(HT):
                    for t in range(NT_I):
                        st = h * S + 2 * t * IT
                        nc.tensor.matmul(
                            pss[t][:],
                            wq[:, h * HD : (h + 1) * HD],
                            xt[:, st : st + 2 * IT : 2],
                            start=(h == 0),
                            stop=(h == HT - 1),
                        )
                for t in range(NT_I):
                    c0 = fq * S_LOC + t * IT
                    rope_evict(
                        rp1, pss[t],
                        qT[0:HH, c0 : c0 + IT], qT[HH:P, c0 : c0 + IT],
                        qcos[:, t * IT : (t + 1) * IT],
                        qsin[:, t * IT : (t + 1) * IT],
                    )

        # ---------------- phase 2: attention --------------------------
        late = body.enter_context(tc.tile_pool(name="late", bufs=1, side="right"))
        yT = late.tile([P, NH * S_LOC], dt.bfloat16)
        # prefetch w_proj during attention
        wp = late.tile([P, HT * H], dt.bfloat16)
        nc.sync.dma_start(wp[:], wp_d.ap())

        with ExitStack() as ph2:
            prp = ph2.enter_context(tc.tile_pool(name="prp", bufs=4))
            dsp = ph2.enter_context(tc.tile_pool(name="dsp", bufs=5))
            recp = ph2.enter_context(tc.tile_pool(name="recp", bufs=2))
            pss_p = ph2.enter_context(tc.tile_pool(name="pssp", bufs=3, space="PSUM"))
            psy = ph2.enter_context(tc.tile_pool(name="psy", bufs=1, space="PSUM"))
            psd = ph2.enter_context(tc.tile_pool(name="psd", bufs=1, space="PSUM"))

            for hq in range(NH):
                kvh = hq // G
                for t in range(NT_I):
                    nj = (t + 1) * JB
                    qsl = qT[:, hq * S_LOC + t * IT : hq * S_LOC + (t + 1) * IT]
                    yps = psy.tile([P, IT], dt.float32, name="yps")
                    dps = psd.tile([P, IT], dt.float32, name="dps")
                    for p in range(nj // 2):
                        j0, j1 = 2 * p, 2 * p + 1
                        c0l = max(0, 64 * (j0 - JB * t))
                        c0r = max(0, 64 * (j1 - JB * t))
                        sps = pss_p.tile([P, 2 * IT], dt.float32, name="sps")
                        nc.tensor.matmul(
                            sps[:, c0l:IT],
                            kT[:, kvh * S + j0 * P : kvh * S + (j0 + 1) * P],
                            qsl[:, c0l:IT],
                            start=True, stop=True,
                        )
                        # right tile packed at column IT (query c0r -> col IT)
                        nc.tensor.matmul(
                            sps[:, IT : 2 * IT - c0r],
                            kT[:, kvh * S + j1 * P : kvh * S + (j1 + 1) * P],
                            qsl[:, c0r:IT],
                            start=True, stop=True,
                        )
                        pr = prp.tile([P, 2 * IT], dt.bfloat16, name="pr")
                        nc.scalar.activation(
                            pr[:, c0l : 2 * IT - c0r],
                            sps[:, c0l : 2 * IT - c0r],
                            AF.Exp, scale=SCALE,
                        )
                        # diagonal-band boundary slivers (self-similar mask)
                        if 0 <= j0 - JB * t < JB:
                            nc.vector.tensor_mul(
                                pr[:, c0l : c0l + 64], pr[:, c0l : c0l + 64],
                                msl[:, 0:64],
                            )
                        if 0 <= j1 - JB * t < JB:
                            nc.vector.tensor_mul(
                                pr[:, IT : IT + 64], pr[:, IT : IT + 64],
                                msl[:, 0:64],
                            )
                        # pair-sum for the denominator
                        ds = dsp.tile([P, IT], dt.bfloat16, name="ds")
                        nc.vector.tensor_add(
                            ds[:, c0r:IT], pr[:, c0r:IT],
                            pr[:, IT : 2 * IT - c0r],
                        )
                        if c0r > c0l:
                            nc.vector.tensor_scalar_mul(
                                ds[:, c0l:c0r], pr[:, c0l:c0r], 1.0
                            )
                        nc.tensor.matmul(
                            yps[:, c0l:IT],
                            vv[:, j0 * KVC + kvh * HD : j0 * KVC + (kvh + 1) * HD],
                            pr[:, c0l:IT],
                            start=(j0 == 0), stop=False,
                            skip_group_check=True,
                        )
                        nc.tensor.matmul(
                            yps[:, c0r:IT],
                            vv[:, j1 * KVC + kvh * HD : j1 * KVC + (kvh + 1) * HD],
                            pr[:, IT : 2 * IT - c0r],
                            start=False, stop=(j1 == nj - 1),
                            skip_group_check=True,
                        )
                        # denominator inline (ones stationary -> column
                        # sums broadcast): keeps the PE from stalling on
                        # a tail chain of DVE pair-sum deps
                        nc.tensor.matmul(
                            dps[:, c0l:IT], onesm[:], ds[:, c0l:IT],
                            start=(p == 0), stop=(p == nj // 2 - 1),
                            skip_group_check=True,
                        )
                    rec = recp.tile([P, IT], dt.float32, name="rec")
                    nc.vector.reciprocal_approx_fast(rec[:], dps[:])
                    nc.vector.tensor_mul(
                        yT[:, hq * S_LOC + t * IT : hq * S_LOC + (t + 1) * IT],
                        yps[:],
                        rec[:],
                    )

        s_act.close()  # free qT / kT / vv before the projection phase

        # ---------------- phase 3: out projection + rmsnorm ------------
        with ExitStack() as ph3:
            outp = ph3.enter_context(tc.tile_pool(name="outp", bufs=2))
            sqp = ph3.enter_context(tc.tile_pool(name="sqp", bufs=2))
            smp = ph3.enter_context(tc.tile_pool(name="smp", bufs=2))
            po = ph3.enter_context(tc.tile_pool(name="po", bufs=8, space="PSUM"))

            for sl in range(S_LOC // P):
                pso = [po.tile([P, OT], dt.float32, name="pso") for _ in range(NO)]
                for h in range(HT):
                    lhs = yT[:, h * S_LOC + sl * P : h * S_LOC + (sl + 1) * P]
                    for o in range(NO):
                        nc.tensor.matmul(
                            pso[o][:],
                            lhs,
                            wp[:, h * H + o * OT : h * H + (o + 1) * OT],
                            start=(h == 0),
                            stop=(h == HT - 1),
                        )
                # rmsnorm straight off PSUM, per OT-chunk, so the out DMA
                # of early chunks overlaps the tail of the row reduction
                sq = sqp.tile([P, OT], dt.float32, name="sq")
                ssqs = smp.tile([P, NO], dt.float32, name="ssqs")
                for o in range(NO):
                    nc.scalar.activation(
                        sq[:], pso[o][:], AF.Square,
                        accum_out=ssqs[:, o : o + 1],
                    )
                ssq = smp.tile([P, 1], dt.float32, name="ssq")
                nc.vector.tensor_reduce(
                    ssq[:], ssqs[:], mybir.AxisListType.X, mybir.AluOpType.add
                )
                rms = smp.tile([P, 1], dt.float32, name="rms")
                nc.scalar.activation(
                    rms[:], ssq[:], AF.Sqrt, bias=epsb[:], scale=1.0 / H
                )
                rr = smp.tile([P, 1], dt.float32, name="rr")
                nc.vector.reciprocal(rr[:], rms[:])
                ot = outp.tile([P, H], dt.float32, name="ot")
                for o in range(NO):
                    nc.vector.tensor_scalar_mul(
                        ot[:, o * OT : (o + 1) * OT], pso[o][:], rr[:]
                    )
                    nc.sync.dma_start(
                        out_d.ap()[sl * P : (sl + 1) * P, o * OT : (o + 1) * OT],
                        ot[:, o * OT : (o + 1) * OT],
                    )

    nc.compile()
    return nc


# ---------------------------------------------------------------- host side
def _host_shared(w_attn, w_proj, norm_w):
    """Core-independent packed tensors."""
    f32 = np.float32

    def perm_halves(w):  # [H, n, HD] even/odd pairs -> halves
        return np.concatenate([w[..., 0::2], w[..., 1::2]], axis=-1)

    wq = perm_halves(w_attn[:, :H].reshape(H, NH, HD))
    # [NH, P, HT*HD]: contiguous per (head, partition)
    wq = np.ascontiguousarray(
        wq.reshape(HT, P, NH, HD).transpose(2, 1, 0, 3).reshape(NH, P, HT * HD)
    ).astype(BF16)
    wk = perm_halves(w_attn[:, H : H + KVC].reshape(H, NKV, HD))
    # [P, NKV*HT*HD]: contiguous per partition
    wk = np.ascontiguousarray(
        wk.reshape(HT, P, NKV, HD).transpose(1, 2, 0, 3).reshape(P, NKV * HT * HD)
    ).astype(BF16)
    # [P, HT*KVC]: contiguous per partition
    wv = np.ascontiguousarray(
        w_attn[:, H + KVC :].reshape(HT, P, KVC).transpose(1, 0, 2).reshape(P, HT * KVC)
    ).astype(BF16)
    # norm_w folded into the projection columns: (y@wp)[s,o]*nw[o]
    wpn = w_proj * norm_w[None, :].astype(f32)
    wp = np.ascontiguousarray(
        wpn.reshape(HT, P, H).transpose(1, 0, 2).reshape(P, HT * H)
    ).astype(BF16)

    p, f = np.meshgrid(np.arange(P), np.arange(64), indexing="ij")
    # self-similar diagonal sliver: parity qp, key order qp-swapped
    msl0 = (2 * f >= p).astype(BF16)                    # qp = 0
    msl1 = (2 * f + 1 >= (p ^ 1)).astype(BF16)          # qp = 1

    return wq, wk, wv, wp, (ustep, vstep0, vstep1)


def _cos_sin(pos):
    f32 = np.float32
    inv = 1.0 / (
        10000.0 ** (np.arange(0, HD, 2, dtype=f32) / f32(HD))
    )
    ang = inv[:, None].astype(f32) * pos[None, :].astype(f32)  # [HH, N]
    c, s = np.cos(ang).astype(BF16), np.sin(ang).astype(BF16)
    return (
        np.ascontiguousarray(np.concatenate([c, c], axis=0)),
        np.ascontiguousarray(np.concatenate([s, s], axis=0)),
    )


def make_in_maps(x, w_attn, w_proj, norm_w):
    x = np.asarray(x, dtype=np.float32)
    w_attn = np.asarray(w_attn, dtype=np.float32)
    w_proj = np.asarray(w_proj, dtype=np.float32)
    norm_w = np.asarray(norm_w, dtype=np.float32)

    wq, wk, wv, wp, (ustep, vstep0, vstep1) = _host_shared(w_attn, w_proj, norm_w)

    kc0, ks0 = _cos_sin(np.arange(S, dtype=np.float32))          # parity 0
    kc1, ks1 = _cos_sin((np.arange(S) ^ 1).astype(np.float32))
    qc0, qs0 = _cos_sin(2.0 * np.arange(S_LOC, dtype=np.float32))
    qc1, qs1 = _cos_sin(2.0 * np.arange(S_LOC, dtype=np.float32) + 1.0)

    in_maps = []
    for c in range(N_CORES):
        b, par = c // 2, c % 2
        xt = x[b].T.astype(BF16)
        if par:
            xt = xt[:, np.arange(S) ^ 1]  # swap adjacent column pairs
        # [P, HT*S]: contiguous per partition
        xt = np.ascontiguousarray(
            xt.reshape(HT, P, S).transpose(1, 0, 2).reshape(P, HT * S)
        )
        in_maps.append(
            {
                "xt": xt,
                "wq": wq,
                "wk": wk,
                "wv": wv,
                "wp": wp,
                "qcos": qc1 if par else qc0,
                "qsin": qs1 if par else qs0,
                "kcos": kc1 if par else kc0,
                "ksin": ks1 if par else ks0,
                "msl": msl1 if par else msl0,
                "nw": nw,
            }
        )
    return in_maps


def assemble_out(results):
    out = np.empty((B, S, H), dtype=np.float32)
    for c in range(N_CORES):
        b, par = c // 2, c % 2
        out[b, par::2, :] = results[c]["out"]
    return out


def kernel(x, w_attn, w_proj, norm_w):
    from concourse import bass_utils

    if "nc" not in _CACHE:
        _CACHE["nc"] = _build_nc()
    nc = _CACHE["nc"]

    in_maps = make_in_maps(x, w_attn, w_proj, norm_w)
    res = bass_utils.run_bass_kernel_spmd(
        nc, in_maps, core_ids=list(range(N_CORES))
    )
    return assemble_out(res.results)


# revision 30
# speedup vs baseline: 1.0064x; 1.0064x over previous
"""Trainium2 Bass kernel for nn_Attention_8366596292664.

Dense transformer block: qkv proj -> RoPE -> GQA causal attention ->
out proj -> RMSNorm.  B=4, S=2048, H=2048, 16 heads (hd=128), 4 KV heads.

Sharding: 8 cores = (4 batches) x (2 interleaved query-row parities).
Core (b, par) computes the full block for query rows {par, par+2, ...} of
batch b.  Interleaving the query rows by parity makes the causal structure
identical on every core, so one SPMD program serves all 8 cores; the
parity enters only through the data (a 1-column roll of x^T, cos/sin
tables, and the output row scatter).

v2 changes vs v1:
  - Trapezoid causal attention: scores/exp/mask/probsV/denominator all
    run on the live column sub-range of each key tile.  The diagonal
    64-column sliver mask is self-similar across tiles, so the mask
    table shrinks to one [128, 64] tile.
  - Startup: x^T loaded in four column blocks + k-proj iterates blocks
    outermost, and dummy warm-up matmuls run during the DMA wait so the
    PE HAM clock-gate is released before real work arrives.
  - w_proj prefetched during attention (kills the phase-3 entry stall).
"""

import numpy as np
import ml_dtypes

BF16 = ml_dtypes.bfloat16

# ---------------------------------------------------------------- config
P = 128          # partitions
HD = 128         # head dim
HH = HD // 2     # rope half
G = 4            # GQA group size

B = 4
S = 2048
H = 2048
N_CORES = 8

NH = H // HD          # 16 q heads
NKV = NH // G         # 4 kv heads
KVC = NKV * HD        # 512 kv columns
HT = H // P           # 16 h-tiles (contraction tiles)
S_LOC = S // 2        # 1024 local q rows per core
IT = 512              # i-tile (queries per score tile, = 1 psum bank fp32)
NT_I = S_LOC // IT    # 2 i-slots
JB = 8                # j-tiles per 1024-key band
OT = 512              # output-proj column tile
NO = H // OT          # 4
NWARM = 260           # PE warm-up matmuls: cover the ~28us startup DMA

RMS_EPS = 1e-6
SCALE = 1.0 / float(np.sqrt(np.float32(HD)))

_CACHE = {}


# ---------------------------------------------------------------- device IR
def _build_nc():
    from contextlib import ExitStack

    import concourse.bacc as bacc
    import concourse.mybir as mybir
    import concourse.tile as tile

    dt = mybir.dt
    AF = mybir.ActivationFunctionType

    nc = bacc.Bacc("TRN2", target_bir_lowering=False, debug=False)

    # host-packed so every DMA row is contiguous per partition (no small
    # packets): wq[f][p] = 2KB, wk/wv[p] = 8KB, wp[p] = 64KB, xt[p] = 64KB
    xt_d = nc.dram_tensor("xt", [P, HT * S], dt.bfloat16, kind="ExternalInput")
    wq_d = nc.dram_tensor("wq", [NH, P, HT * HD], dt.bfloat16, kind="ExternalInput")
    wk_d = nc.dram_tensor("wk", [P, NKV * HT * HD], dt.bfloat16, kind="ExternalInput")
    wv_d = nc.dram_tensor("wv", [P, HT * KVC], dt.bfloat16, kind="ExternalInput")
    wp_d = nc.dram_tensor("wp", [P, HT * H], dt.bfloat16, kind="ExternalInput")
    qcos_d = nc.dram_tensor("qcos", [P, S_LOC], dt.bfloat16, kind="ExternalInput")
    qsin_d = nc.dram_tensor("qsin", [P, S_LOC], dt.bfloat16, kind="ExternalInput")
    kcos_d = nc.dram_tensor("kcos", [P, S], dt.bfloat16, kind="ExternalInput")
    ksin_d = nc.dram_tensor("ksin", [P, S], dt.bfloat16, kind="ExternalInput")
    msl_d = nc.dram_tensor("msl", [P, 64], dt.bfloat16, kind="ExternalInput")
    out_d = nc.dram_tensor("out", [S_LOC, H], dt.float32, kind="ExternalOutput")

    with tile.TileContext(nc) as tc, ExitStack() as body:
        const = body.enter_context(tc.tile_pool(name="const", bufs=1))
        qcos = const.tile([P, S_LOC], dt.bfloat16)
        qsin = const.tile([P, S_LOC], dt.bfloat16)
        kcos = const.tile([P, S], dt.bfloat16)
        ksin = const.tile([P, S], dt.bfloat16)
        onesm = const.tile([P, P], dt.bfloat16)
        nc.vector.memset(onesm[:], 1.0)
        msl = const.tile([P, 64], dt.bfloat16)
        epsb = const.tile([P, 1], dt.float32)
        nc.vector.memset(epsb[:], RMS_EPS)
        nc.sync.dma_start(msl[:], msl_d.ap())
        nc.sync.dma_start(qcos[:], qcos_d.ap())
        nc.sync.dma_start(qsin[:], qsin_d.ap())
        nc.sync.dma_start(kcos[:], kcos_d.ap())
        nc.sync.dma_start(ksin[:], ksin_d.ap())

        s_act = body.enter_context(ExitStack())
        act = s_act.enter_context(tc.tile_pool(name="act", bufs=1))
        qT = act.tile([P, NH * S_LOC], dt.bfloat16)
        kT = act.tile([P, NKV * S], dt.bfloat16)
        vv = act.tile([P, (S // P) * KVC], dt.bfloat16)

        def rope_evict(rpool, ps, dst_lo, dst_hi, cs, sn):
            # dst_lo = ps_lo*cos - ps_hi*sin ; dst_hi = ps_hi*cos + ps_lo*sin
            stg = rpool.tile([P, IT], dt.bfloat16, name="rstg")
            nc.scalar.activation(stg[:], ps[:], AF.Copy)
            t1 = rpool.tile([HH, IT], dt.bfloat16, name="rt1")
            t2 = rpool.tile([HH, IT], dt.bfloat16, name="rt2")
            nc.vector.tensor_mul(t1[:], stg[0:HH, :], cs[0:HH, :])
            nc.vector.tensor_mul(t2[:], stg[HH:P, :], sn[HH:P, :])
            nc.vector.tensor_sub(dst_lo, t1[:], t2[:])
            nc.vector.tensor_mul(t1[:], stg[HH:P, :], cs[HH:P, :])
            nc.vector.tensor_mul(t2[:], stg[0:HH, :], sn[0:HH, :])
            nc.vector.tensor_add(dst_hi, t1[:], t2[:])

        # ---------------- phase 1: qkv projection + rope --------------
        with ExitStack() as ph1:
            xp = ph1.enter_context(tc.tile_pool(name="xp", bufs=1))
            xt = xp.tile([P, HT * S], dt.bfloat16)

            wkp = ph1.enter_context(tc.tile_pool(name="wkp", bufs=1))
            wk = wkp.tile([P, NKV * HT * HD], dt.bfloat16)
            nc.sync.dma_start(wk[:], wk_d.ap())
            # x^T in four column blocks so the first k-proj tile can start
            # after ~3MB instead of ~13MB of DMA.
            xt_r = xt[:].rearrange("p (t s) -> p t s", t=HT)
            xd_r = xt_d.ap().rearrange("p (t s) -> p t s", t=HT)
            for sc in range(S // IT):
                nc.sync.dma_start(
                    xt_r[:, :, sc * IT : (sc + 1) * IT],
                    xd_r[:, :, sc * IT : (sc + 1) * IT],
                )

            wvp = ph1.enter_context(tc.tile_pool(name="wvp", bufs=1))
            wv = wvp.tile([P, HT * KVC], dt.bfloat16)
            nc.sync.dma_start(wv[:], wv_d.ap())

            wqp = ph1.enter_context(tc.tile_pool(name="wqp", bufs=2))
            rp1 = ph1.enter_context(tc.tile_pool(name="rp1", bufs=2))
            psq = ph1.enter_context(tc.tile_pool(name="psq", bufs=4, space="PSUM"))
            psk = ph1.enter_context(tc.tile_pool(name="psk", bufs=2, space="PSUM"))
            psv = ph1.enter_context(tc.tile_pool(name="psv", bufs=2, space="PSUM"))

            # PE warm-up: release the HAM clock gate while startup DMAs
            # are in flight (no data deps -> runs immediately).
            wps = psk.tile([P, IT], dt.float32, name="kps")
            for _ in range(NWARM):
                nc.tensor.matmul(wps[:, 0:P], onesm[:], onesm[:], start=True, stop=True)

            # k projection (sc-block outermost: only needs xt block sc) + rope
            for sc in range(S // IT):
                for fk in range(NKV):
                    ps = psk.tile([P, IT], dt.float32, name="kps")
                    for h in range(HT):
                        nc.tensor.matmul(
                            ps[:],
                            wk[:, fk * H + h * HD : fk * H + (h + 1) * HD],
                            xt[:, h * S + sc * IT : h * S + (sc + 1) * IT],
                            start=(h == 0),
                            stop=(h == HT - 1),
                        )
                    c0 = fk * S + sc * IT
                    rope_evict(
                        rp1, ps,
                        kT[0:HH, c0 : c0 + IT], kT[HH:P, c0 : c0 + IT],
                        kcos[:, sc * IT : (sc + 1) * IT],
                        ksin[:, sc * IT : (sc + 1) * IT],
                    )

            # v projection (natural [s, f] layout)
            for sv in range(S // P):
                ps = psv.tile([P, KVC], dt.float32, name="vps")
                for h in range(HT):
                    nc.tensor.matmul(
                        ps[:],
                        xt[:, h * S + sv * P : h * S + (sv + 1) * P],
                        wv[:, h * KVC : (h + 1) * KVC],
                        start=(h == 0),
                        stop=(h == HT - 1),
                    )
                nc.scalar.activation(
                    vv[:, sv * KVC : (sv + 1) * KVC], ps[:], AF.Copy
                )

            # q projection (local rows, stride-2 reads of x^T) + rope.
            for fq in range(NH):
                wq = wqp.tile([P, HT * HD], dt.bfloat16, name="wqt")
                if fq == 0:
                    nc.vector.tensor_scalar_mul(wq[:, 0:1], wv[:, 0:1], 1.0)
                nc.sync.dma_start(wq[:], wq_d.ap()[fq])
                pss = [psq.tile([P, IT], dt.float32, name="qps") for _ in range(NT_I)]
                for h in range(HT):
                    for t in range(NT_I):
                        st = h * S + 2 * t * IT
                        nc.tensor.matmul(
                            pss[t][:],
                            wq[:, h * HD : (h + 1) * HD],
                            xt[:, st : st + 2 * IT : 2],
                            start=(h == 0),
                            stop=(h == HT - 1),
                        )
                for t in range(NT_I):
                    c0 = fq * S_LOC + t * IT
                    rope_evict(
                        rp1, pss[t],
                        qT[0:HH, c0 : c0 + IT], qT[HH:P, c0 : c0 + IT],
                        qcos[:, t * IT : (t + 1) * IT],
                        qsin[:, t * IT : (t + 1) * IT],
                    )

        # ---------------- phase 2: attention --------------------------
        late = body.enter_context(tc.tile_pool(name="late", bufs=1, side="right"))
        yT = late.tile([P, NH * S_LOC], dt.bfloat16)
        # prefetch w_proj during attention
        wp = late.tile([P, HT * H], dt.bfloat16)
        nc.sync.dma_start(wp[:], wp_d.ap())

        with ExitStack() as ph2:
            prp = ph2.enter_context(tc.tile_pool(name="prp", bufs=4))
            dsp = ph2.enter_context(tc.tile_pool(name="dsp", bufs=5))
            recp = ph2.enter_context(tc.tile_pool(name="recp", bufs=2))
            pss_p = ph2.enter_context(tc.tile_pool(name="pssp", bufs=3, space="PSUM"))
            psy = ph2.enter_context(tc.tile_pool(name="psy", bufs=1, space="PSUM"))
            psd = ph2.enter_context(tc.tile_pool(name="psd", bufs=1, space="PSUM"))

            for hq in range(NH):
                kvh = hq // G
                for t in range(NT_I):
                    nj = (t + 1) * JB
                    qsl = qT[:, hq * S_LOC + t * IT : hq * S_LOC + (t + 1) * IT]
                    yps = psy.tile([P, IT], dt.float32, name="yps")
                    dps = psd.tile([P, IT], dt.float32, name="dps")
                    for p in range(nj // 2):
                        j0, j1 = 2 * p, 2 * p + 1
                        c0l = max(0, 64 * (j0 - JB * t))
                        c0r = max(0, 64 * (j1 - JB * t))
                        sps = pss_p.tile([P, 2 * IT], dt.float32, name="sps")
                        nc.tensor.matmul(
                            sps[:, c0l:IT],
                            kT[:, kvh * S + j0 * P : kvh * S + (j0 + 1) * P],
                            qsl[:, c0l:IT],
                            start=True, stop=True,
                        )
                        # right tile packed at column IT (query c0r -> col IT)
                        nc.tensor.matmul(
                            sps[:, IT : 2 * IT - c0r],
                            kT[:, kvh * S + j1 * P : kvh * S + (j1 + 1) * P],
                            qsl[:, c0r:IT],
                            start=True, stop=True,
                        )
                        pr = prp.tile([P, 2 * IT], dt.bfloat16, name="pr")
                        nc.scalar.activation(
                            pr[:, c0l : 2 * IT - c0r],
                            sps[:, c0l : 2 * IT - c0r],
                            AF.Exp, scale=SCALE,
                        )
                        # diagonal-band boundary slivers (self-similar mask)
                        if 0 <= j0 - JB * t < JB:
                            nc.vector.tensor_mul(
                                pr[:, c0l : c0l + 64], pr[:, c0l : c0l + 64],
                                msl[:, 0:64],
                            )
                        if 0 <= j1 - JB * t < JB:
                            nc.vector.tensor_mul(
                                pr[:, IT : IT + 64], pr[:, IT : IT + 64],
                                msl[:, 0:64],
                            )
                        # pair-sum for the denominator
                        ds = dsp.tile([P, IT], dt.bfloat16, name="ds")
                        nc.vector.tensor_add(
                            ds[:, c0r:IT], pr[:, c0r:IT],
                            pr[:, IT : 2 * IT - c0r],
                        )
                        if c0r > c0l:
                            nc.vector.tensor_scalar_mul(
                                ds[:, c0l:c0r], pr[:, c0l:c0r], 1.0
                            )
                        nc.tensor.matmul(
                            yps[:, c0l:IT],
                            vv[:, j0 * KVC + kvh * HD : j0 * KVC + (kvh + 1) * HD],
                            pr[:, c0l:IT],
                            start=(j0 == 0), stop=False,
                            skip_group_check=True,
                        )
                        nc.tensor.matmul(
                            yps[:, c0r:IT],
                            vv[:, j1 * KVC + kvh * HD : j1 * KVC + (kvh + 1) * HD],
                            pr[:, IT : 2 * IT - c0r],
                            start=False, stop=(j1 == nj - 1),
                            skip_group_check=True,
                        )
                        # denominator inline (ones stationary -> column
                        # sums broadcast): keeps the PE from stalling on
                        # a tail chain of DVE pair-sum deps
                        nc.tensor.matmul(
                            dps[:, c0l:IT], onesm[:], ds[:, c0l:IT],
                            start=(p == 0), stop=(p == nj // 2 - 1),
                            skip_group_check=True,
                        )
                    rec = recp.tile([P, IT], dt.float32, name="rec")
                    nc.vector.reciprocal_approx_fast(rec[:], dps[:])
                    nc.vector.tensor_mul(
                        yT[:, hq * S_LOC + t * IT : hq * S_LOC + (t + 1) * IT],
                        yps[:],
                        rec[:],
                    )

        s_act.close()  # free qT / kT / vv before the projection phase

        # ---------------- phase 3: out projection + rmsnorm ------------
        with ExitStack() as ph3:
            outp = ph3.enter_context(tc.tile_pool(name="outp", bufs=2))
            sqp = ph3.enter_context(tc.tile_pool(name="sqp", bufs=2))
            smp = ph3.enter_context(tc.tile_pool(name="smp", bufs=2))
            po = ph3.enter_context(tc.tile_pool(name="po", bufs=8, space="PSUM"))

            for sl in range(S_LOC // P):
                pso = [po.tile([P, OT], dt.float32, name="pso") for _ in range(NO)]
                for o in range(NO):
                    for h in range(HT):
                        nc.tensor.matmul(
                            pso[o][:],
                            yT[:, h * S_LOC + sl * P : h * S_LOC + (sl + 1) * P],
                            wp[:, h * H + o * OT : h * H + (o + 1) * OT],
                            start=(h == 0),
                            stop=(h == HT - 1),
                        )
                # rmsnorm straight off PSUM, per OT-chunk, so the out DMA
                # of early chunks overlaps the tail of the row reduction
                sq = sqp.tile([P, OT], dt.float32, name="sq")
                ssqs = smp.tile([P, NO], dt.float32, name="ssqs")
                for o in range(NO):
                    nc.scalar.activation(
                        sq[:], pso[o][:], AF.Square,
                        accum_out=ssqs[:, o : o + 1],
                    )
                ssq = smp.tile([P, 1], dt.float32, name="ssq")
                nc.vector.tensor_reduce(
                    ssq[:], ssqs[:], mybir.AxisListType.X, mybir.AluOpType.add
                )
                rms = smp.tile([P, 1], dt.float32, name="rms")
                nc.scalar.activation(
                    rms[:], ssq[:], AF.Sqrt, bias=epsb[:], scale=1.0 / H
                )
                rr = smp.tile([P, 1], dt.float32, name="rr")
                nc.vector.reciprocal(rr[:], rms[:])
                ot = outp.tile([P, H], dt.float32, name="ot")
                for o in range(NO):
                    nc.vector.tensor_scalar_mul(
                        ot[:, o * OT : (o + 1) * OT], pso[o][:], rr[:]
                    )
                    nc.sync.dma_start(
                        out_d.ap()[sl * P : (sl + 1) * P, o * OT : (o + 1) * OT],
                        ot[:, o * OT : (o + 1) * OT],
                    )

    nc.compile()
    return nc


# ---------------------------------------------------------------- host side
def _host_shared(w_attn, w_proj, norm_w):
    """Core-independent packed tensors."""
    f32 = np.float32

    def perm_halves(w):  # [H, n, HD] even/odd pairs -> halves
        return np.concatenate([w[..., 0::2], w[..., 1::2]], axis=-1)

    wq = perm_halves(w_attn[:, :H].reshape(H, NH, HD))
    # [NH, P, HT*HD]: contiguous per (head, partition)
    wq = np.ascontiguousarray(
        wq.reshape(HT, P, NH, HD).transpose(2, 1, 0, 3).reshape(NH, P, HT * HD)
    ).astype(BF16)
    wk = perm_halves(w_attn[:, H : H + KVC].reshape(H, NKV, HD))
    # [P, NKV*HT*HD]: contiguous per partition
    wk = np.ascontiguousarray(
        wk.reshape(HT, P, NKV, HD).transpose(1, 2, 0, 3).reshape(P, NKV * HT * HD)
    ).astype(BF16)
    # [P, HT*KVC]: contiguous per partition
    wv = np.ascontiguousarray(
        w_attn[:, H + KVC :].reshape(HT, P, KVC).transpose(1, 0, 2).reshape(P, HT * KVC)
    ).astype(BF16)
    # norm_w folded into the projection columns: (y@wp)[s,o]*nw[o]
    wpn = w_proj * norm_w[None, :].astype(f32)
    wp = np.ascontiguousarray(
        wpn.reshape(HT, P, H).transpose(1, 0, 2).reshape(P, HT * H)
    ).astype(BF16)

    p, f = np.meshgrid(np.arange(P), np.arange(64), indexing="ij")
    # self-similar diagonal sliver: parity qp, key order qp-swapped
    msl0 = (2 * f >= p).astype(BF16)                    # qp = 0
    msl1 = (2 * f + 1 >= (p ^ 1)).astype(BF16)          # qp = 1

    return wq, wk, wv, wp, (ustep, vstep0, vstep1)


def _cos_sin(pos):
    f32 = np.float32
    inv = 1.0 / (
        10000.0 ** (np.arange(0, HD, 2, dtype=f32) / f32(HD))
    )
    ang = inv[:, None].astype(f32) * pos[None, :].astype(f32)  # [HH, N]
    c, s = np.cos(ang).astype(BF16), np.sin(ang).astype(BF16)
    return (
        np.ascontiguousarray(np.concatenate([c, c], axis=0)),
        np.ascontiguousarray(np.concatenate([s, s], axis=0)),
    )


def make_in_maps(x, w_attn, w_proj, norm_w):
    x = np.asarray(x, dtype=np.float32)
    w_attn = np.asarray(w_attn, dtype=np.float32)
    w_proj = np.asarray(w_proj, dtype=np.float32)
    norm_w = np.asarray(norm_w, dtype=np.float32)

    wq, wk, wv, wp, (ustep, vstep0, vstep1) = _host_shared(w_attn, w_proj, norm_w)

    kc0, ks0 = _cos_sin(np.arange(S, dtype=np.float32))          # parity 0
    kc1, ks1 = _cos_sin((np.arange(S) ^ 1).astype(np.float32))
    qc0, qs0 = _cos_sin(2.0 * np.arange(S_LOC, dtype=np.float32))
    qc1, qs1 = _cos_sin(2.0 * np.arange(S_LOC, dtype=np.float32) + 1.0)

    in_maps = []
    for c in range(N_CORES):
        b, par = c // 2, c % 2
        xt = x[b].T.astype(BF16)
        if par:
            xt = xt[:, np.arange(S) ^ 1]  # swap adjacent column pairs
        # [P, HT*S]: contiguous per partition
        xt = np.ascontiguousarray(
            xt.reshape(HT, P, S).transpose(1, 0, 2).reshape(P, HT * S)
        )
        in_maps.append(
            {
                "xt": xt,
                "wq": wq,
                "wk": wk,
                "wv": wv,
                "wp": wp,
                "qcos": qc1 if par else qc0,
                "qsin": qs1 if par else qs0,
                "kcos": kc1 if par else kc0,
                "ksin": ks1 if par else ks0,
                "msl": msl1 if par else msl0,
                "nw": nw,
            }
        )
    return in_maps


def assemble_out(results):
    out = np.empty((B, S, H), dtype=np.float32)
    for c in range(N_CORES):
        b, par = c // 2, c % 2
        out[b, par::2, :] = results[c]["out"]
    return out


def kernel(x, w_attn, w_proj, norm_w):
    from concourse import bass_utils

    if "nc" not in _CACHE:
        _CACHE["nc"] = _build_nc()
    nc = _CACHE["nc"]

    in_maps = make_in_maps(x, w_attn, w_proj, norm_w)
    res = bass_utils.run_bass_kernel_spmd(
        nc, in_maps, core_ids=list(range(N_CORES))
    )
    return assemble_out(res.results)
